# revision 2
# baseline (speedup 1.0000x reference)
"""Per-channel Linear(seq->pred) over channels, 8-core channel-parallel Trainium2 kernel.

Math: y[b,p,c] = sum_s x[b,s,c] * W[c,p,s] + bias[c,p]

Strategy (bf16 HBM traffic -- the kernel is DMA-bound, W is read exactly once):
  - Shard channels C=321 across 8 cores (pad to 328 = 8*41).
  - Host-side re-layout into bf16 (contraction padded to 726 = 6*121 rows):
      row s<720 = data, row 720 = bias (x row = 1.0), rows 721+ = 0,
    so bias is folded into the contraction and K splits into 6 chunks of 121.
  - W is pre-chunked on host to wq[pair, 121, (c2 k), P] so each pair's W
    load is a single fully-contiguous 2.1MB DMA (17.3KB per partition row).
  - All x is preloaded to SBUF once (4MB bf16); per-pair slices.
  - Per channel: Y_c[b,p] = sum_k xT_chunk[k].T @ wT_chunk[k], accumulated in
    PSUM f32 over the 6 K-chunks; rhs streamed as N = 512 + 208 (PSUM bank).
  - Two channels share one PSUM tile (output partitions 0:64 / 64:128); the
    21st pair carries the lone 41st channel (half the DMA, half the matmuls).
  - W streams on the sync HWDGE ring; x loads + y stores go on the scalar
    (ACT) HWDGE ring so they stay off the W critical path.
  - Result copied PSUM->SBUF with f32->bf16 cast (DVE + ACT split), y written
    back as bf16 and upcast on host.
"""

import numpy as np
import ml_dtypes

import concourse.bacc as bacc
import concourse.mybir as mybir
import concourse.tile as tile
from concourse.bass_utils import run_bass_kernel_spmd

F32 = mybir.dt.float32
BF16 = mybir.dt.bfloat16
NPBF16 = ml_dtypes.bfloat16

B = 64          # batch
S = 720         # seq_len (contraction)
P = 720         # pred_len
C = 321         # channels
N_CORES = 8
CL = 41         # channels per core; 8*41 = 328 >= 321
CPAD = N_CORES * CL
NPAIR = (CL + 1) // 2  # 21 channel pairs per core (last one is a single)
KCH = 121       # K-chunk rows
NKCH = 6        # chunks per channel
SPAD = KCH * NKCH  # 726 = 720 data + bias + 5 zero
NSPLIT = 512    # first matmul N (PSUM bank holds 512 f32)

_CACHE: dict = {}


def _build_module():
    nc = bacc.Bacc("TRN2", target_bir_lowering=False, debug=False,
                   num_devices=N_CORES)
    # W pre-chunked: wq[j, s, c2*NKCH+k, p] = W[c(j,c2), p, k*KCH+s] (+bias row)
    wq = nc.dram_tensor("wq", [NPAIR, KCH, 2 * NKCH, P], BF16,
                        kind="ExternalInput").ap()
    # x pre-chunked: xq[s, j, c2, k, b]
    xq = nc.dram_tensor("xq", [KCH, NPAIR, 2, NKCH, B], BF16,
                        kind="ExternalInput").ap()
    y = nc.dram_tensor("y", [CL, B, P], BF16, kind="ExternalOutput").ap()

    with tile.TileContext(nc) as tc:
        with (
            tc.tile_pool(name="xp", bufs=1) as xp,
            tc.tile_pool(name="wp", bufs=4) as wp,
            tc.tile_pool(name="pp", bufs=3, space="PSUM") as pp,
            tc.tile_pool(name="op", bufs=3) as op,
        ):
            # preload all of x (per-pair DMAs so pair j only waits on its slice)
            xall = xp.tile([KCH, NPAIR, 2, NKCH, B], BF16, name="xall")
            for j in range(NPAIR):
                nc.scalar.dma_start(xall[:, j], xq[:, j])

            for j in range(NPAIR):
                pair = 2 if j < NPAIR - 1 else 1
                wbig = wp.tile([KCH, 2 * NKCH, P], BF16, name=f"w{j}",
                               tag="wbig")
                if pair == 2:
                    nc.sync.dma_start(wbig[:], wq[j])
                else:
                    nc.sync.dma_start(wbig[:, 0:NKCH], wq[j, :, 0:NKCH])
                ps = pp.tile([pair * B, P], F32, name=f"ps{j}", tag="ps")
                for k in range(NKCH):
                    st, sp = (k == 0), (k == NKCH - 1)
                    for half in range(pair):
                        lhsT = xall[:, j, half, k, :]
                        ck = half * NKCH + k
                        prow = half * B
                        nc.tensor.matmul(ps[prow:prow + B, 0:NSPLIT],
                                         lhsT, wbig[:, ck, 0:NSPLIT],
                                         start=st, stop=sp)
                        nc.tensor.matmul(ps[prow:prow + B, NSPLIT:P],
                                         lhsT, wbig[:, ck, NSPLIT:P],
                                         start=st, stop=sp)
                out = op.tile([pair * B, P], BF16, name=f"o{j}", tag="out")
                nc.vector.tensor_copy(out[:, 0:NSPLIT], ps[:, 0:NSPLIT])
                nc.scalar.copy(out[:, NSPLIT:P], ps[:, NSPLIT:P])
                nc.scalar.dma_start(
                    y[2 * j:2 * j + pair].rearrange("c b p -> (c b) p"),
                    out[:])

    nc.compile()
    return nc


def _get_module():
    if "nc" not in _CACHE:
        _CACHE["nc"] = _build_module()
    return _CACHE["nc"]


def _prep_inputs(x, W, b):
    # channel-major stacks with the bias folded in as contraction row 720
    wt = np.zeros((CPAD, SPAD, P), dtype=NPBF16)
    wt[:C, :S, :] = W.transpose(0, 2, 1).astype(NPBF16)
    wt[:C, S, :] = b.astype(NPBF16)
    xt = np.zeros((CPAD, SPAD, B), dtype=NPBF16)
    xt[:C, :S, :] = x.transpose(2, 1, 0).astype(NPBF16)
    xt[:C, S, :] = np.asarray(1.0, dtype=NPBF16)
    in_maps = []
    for i in range(N_CORES):
        wc = wt[i * CL:(i + 1) * CL]
        xc = xt[i * CL:(i + 1) * CL]
        wqa = np.zeros((NPAIR, KCH, 2 * NKCH, P), dtype=NPBF16)
        wqa[:NPAIR - 1] = (wc[:2 * (NPAIR - 1)]
                           .reshape(NPAIR - 1, 2, NKCH, KCH, P)
                           .transpose(0, 3, 1, 2, 4)
                           .reshape(NPAIR - 1, KCH, 2 * NKCH, P))
        wqa[NPAIR - 1, :, :NKCH] = (wc[CL - 1].reshape(NKCH, KCH, P)
                                    .transpose(1, 0, 2))
        xqa = np.zeros((KCH, NPAIR, 2, NKCH, B), dtype=NPBF16)
        xqa[:, :NPAIR - 1] = (xc[:2 * (NPAIR - 1)]
                              .reshape(NPAIR - 1, 2, NKCH, KCH, B)
                              .transpose(3, 0, 1, 2, 4))
        xqa[:, NPAIR - 1, 0] = (xc[CL - 1].reshape(NKCH, KCH, B)
                                .transpose(1, 0, 2))
        in_maps.append({
            "wq": np.ascontiguousarray(wqa),
            "xq": np.ascontiguousarray(xqa),
        })
    return in_maps


def _gather(results):
    ys = np.concatenate([results[i]["y"] for i in range(N_CORES)], axis=0)
    return np.ascontiguousarray(ys[:C].transpose(1, 2, 0)).astype(np.float32)


def run(x, W, b, **run_kwargs):
    """Full pipeline, returns (output, BassKernelResults)."""
    nc = _get_module()
    in_maps = _prep_inputs(np.asarray(x), np.asarray(W), np.asarray(b))
    res = run_bass_kernel_spmd(nc, in_maps, list(range(N_CORES)), **run_kwargs)
    return _gather(res.results), res


def kernel(x, W, b):
    out, _ = run(x, W, b)
    return out


# revision 3
# speedup vs baseline: 1.6559x; 1.6559x over previous
"""Per-channel Linear(seq->pred) over channels, 8-core channel-parallel Trainium2 kernel.

Math: y[b,p,c] = sum_s x[b,s,c] * W[c,p,s] + bias[c,p]

Strategy (bf16 HBM traffic; the kernel is SDMA-engine-bound, W is read once):
  - Shard channels C=321 across 8 cores (pad to 328 = 8*41).
  - Contraction rows: [0..719] = data, row 720 = bias (x row = 1.0),
    rows 721..735 = zero pad -> SPAD=736 = 5*128 + 96. K-chunks of 128 rows
    (and one 96-row tail) keep every DMA's partition count a multiple of 16,
    which is what fans descriptors across all 16 SDMA engines (121-row DMAs
    only engage 11 engines - measured).
  - W pre-chunked on host so each pair's W loads are fully contiguous DMAs
    (17.3KB per partition row); x preloaded to SBUF in two whole-tensor DMAs.
  - Per channel: Y_c[b,p] = sum_k xT_chunk[k].T @ wT_chunk[k], PSUM f32
    accumulation; rhs streamed as N = 512 + 208 (PSUM bank size).
  - Two channels share one PSUM tile (output partitions 0:64 / 64:128); the
    21st pair carries the lone 41st channel (half DMA, half matmuls).
  - W DMAs alternate between the two HWDGE rings (sync / scalar) so both
    descriptor generators keep all 16 SDMA engines fed; x goes on sync at
    the start, y stores on scalar.
  - Result copied PSUM->SBUF with f32->bf16 cast (DVE + ACT split), y written
    back as bf16 and upcast on host.
"""

import numpy as np
import ml_dtypes

import concourse.bacc as bacc
import concourse.mybir as mybir
import concourse.tile as tile
from concourse.bass_utils import run_bass_kernel_spmd

F32 = mybir.dt.float32
BF16 = mybir.dt.bfloat16
NPBF16 = ml_dtypes.bfloat16

B = 64          # batch
S = 720         # seq_len (contraction)
P = 720         # pred_len
C = 321         # channels
N_CORES = 8
CL = 41         # channels per core; 8*41 = 328 >= 321
CPAD = N_CORES * CL
NPAIR = (CL + 1) // 2  # 21 channel pairs per core (last one is a single)
KCH = 128       # K-chunk rows (full chunks)
NKA = 5         # full 128-row chunks
KB = 96         # tail chunk rows (80 data + bias + 15 zero)
SPAD = NKA * KCH + KB  # 736
NSPLIT = 512    # first matmul N (PSUM bank holds 512 f32)

_CACHE: dict = {}


def _build_module():
    nc = bacc.Bacc("TRN2", target_bir_lowering=False, debug=False,
                   num_devices=N_CORES)
    # W pre-chunked: wqa[j, s, c2*NKA+k, p] = W[c(j,c2), p, k*KCH+s]
    wqa = nc.dram_tensor("wqa", [NPAIR, KCH, 2 * NKA, P], BF16,
                         kind="ExternalInput").ap()
    # tail chunk (data rows 640:720 + bias + zeros)
    wqb = nc.dram_tensor("wqb", [NPAIR, KB, 2, P], BF16,
                         kind="ExternalInput").ap()
    # x pre-chunked: xqa[s, j, c2, k, b], xqb[s, j, c2, b]
    xqa = nc.dram_tensor("xqa", [KCH, NPAIR, 2, NKA, B], BF16,
                         kind="ExternalInput").ap()
    xqb = nc.dram_tensor("xqb", [KB, NPAIR, 2, B], BF16,
                         kind="ExternalInput").ap()
    y = nc.dram_tensor("y", [CL, B, P], BF16, kind="ExternalOutput").ap()

    with tile.TileContext(nc) as tc:
        with (
            tc.tile_pool(name="xp", bufs=1) as xp,
            tc.tile_pool(name="wpa", bufs=5) as wpa,
            tc.tile_pool(name="wpb", bufs=5) as wpb,
            tc.tile_pool(name="pp", bufs=3, space="PSUM") as pp,
            tc.tile_pool(name="op", bufs=3) as op,
        ):
            xalla = xp.tile([KCH, NPAIR, 2, NKA, B], BF16, name="xalla")
            xallb = xp.tile([KB, NPAIR, 2, B], BF16, name="xallb")
            nc.sync.dma_start(xalla[:], xqa[:])
            nc.sync.dma_start(xallb[:], xqb[:])

            for j in range(NPAIR):
                pair = 2 if j < NPAIR - 1 else 1
                ring = nc.sync if j % 2 == 0 else nc.scalar
                wba = wpa.tile([KCH, 2 * NKA, P], BF16, name=f"wa{j}",
                               tag="wa")
                wbb = wpb.tile([KB, 2, P], BF16, name=f"wb{j}", tag="wb")
                if pair == 2:
                    ring.dma_start(wba[:], wqa[j])
                    ring.dma_start(wbb[:], wqb[j])
                else:
                    ring.dma_start(wba[:, 0:NKA], wqa[j, :, 0:NKA])
                    ring.dma_start(wbb[:, 0:1], wqb[j, :, 0:1])
                ps = pp.tile([pair * B, P], F32, name=f"ps{j}", tag="ps")
                for k in range(NKA + 1):
                    st, sp = (k == 0), (k == NKA)
                    for half in range(pair):
                        prow = half * B
                        if k < NKA:
                            lhsT = xalla[:, j, half, k, :]
                            rhs = wba[:, half * NKA + k]
                        else:
                            lhsT = xallb[:, j, half, :]
                            rhs = wbb[:, half]
                        nc.tensor.matmul(ps[prow:prow + B, 0:NSPLIT],
                                         lhsT, rhs[:, 0:NSPLIT],
                                         start=st, stop=sp)
                        nc.tensor.matmul(ps[prow:prow + B, NSPLIT:P],
                                         lhsT, rhs[:, NSPLIT:P],
                                         start=st, stop=sp)
                out = op.tile([pair * B, P], BF16, name=f"o{j}", tag="out")
                nc.vector.tensor_copy(out[:, 0:NSPLIT], ps[:, 0:NSPLIT])
                nc.scalar.copy(out[:, NSPLIT:P], ps[:, NSPLIT:P])
                nc.scalar.dma_start(
                    y[2 * j:2 * j + pair].rearrange("c b p -> (c b) p"),
                    out[:])

    nc.compile()
    return nc


def _get_module():
    if "nc" not in _CACHE:
        _CACHE["nc"] = _build_module()
    return _CACHE["nc"]


def _prep_inputs(x, W, b):
    # channel-major stacks, bias folded in as contraction row 720
    wt = np.zeros((CPAD, SPAD, P), dtype=NPBF16)
    wt[:C, :S, :] = W.transpose(0, 2, 1).astype(NPBF16)
    wt[:C, S, :] = b.astype(NPBF16)
    xt = np.zeros((CPAD, SPAD, B), dtype=NPBF16)
    xt[:C, :S, :] = x.transpose(2, 1, 0).astype(NPBF16)
    xt[:C, S, :] = np.asarray(1.0, dtype=NPBF16)
    nfull = 2 * (NPAIR - 1)  # 40 paired channels per core
    in_maps = []
    for i in range(N_CORES):
        wc = wt[i * CL:(i + 1) * CL]
        xc = xt[i * CL:(i + 1) * CL]
        wqa = np.zeros((NPAIR, KCH, 2 * NKA, P), dtype=NPBF16)
        wqa[:NPAIR - 1] = (wc[:nfull, :NKA * KCH]
                           .reshape(NPAIR - 1, 2, NKA, KCH, P)
                           .transpose(0, 3, 1, 2, 4)
                           .reshape(NPAIR - 1, KCH, 2 * NKA, P))
        wqa[NPAIR - 1, :, :NKA] = (wc[CL - 1, :NKA * KCH]
                                   .reshape(NKA, KCH, P).transpose(1, 0, 2))
        wqb = np.zeros((NPAIR, KB, 2, P), dtype=NPBF16)
        wqb[:NPAIR - 1] = (wc[:nfull, NKA * KCH:]
                           .reshape(NPAIR - 1, 2, KB, P)
                           .transpose(0, 2, 1, 3))
        wqb[NPAIR - 1, :, 0] = wc[CL - 1, NKA * KCH:]
        xqa = np.zeros((KCH, NPAIR, 2, NKA, B), dtype=NPBF16)
        xqa[:, :NPAIR - 1] = (xc[:nfull, :NKA * KCH]
                              .reshape(NPAIR - 1, 2, NKA, KCH, B)
                              .transpose(3, 0, 1, 2, 4))
        xqa[:, NPAIR - 1, 0] = (xc[CL - 1, :NKA * KCH]
                                .reshape(NKA, KCH, B).transpose(1, 0, 2))
        xqb = np.zeros((KB, NPAIR, 2, B), dtype=NPBF16)
        xqb[:, :NPAIR - 1] = (xc[:nfull, NKA * KCH:]
                              .reshape(NPAIR - 1, 2, KB, B)
                              .transpose(2, 0, 1, 3))
        xqb[:, NPAIR - 1, 0] = xc[CL - 1, NKA * KCH:]
        in_maps.append({
            "wqa": np.ascontiguousarray(wqa),
            "wqb": np.ascontiguousarray(wqb),
            "xqa": np.ascontiguousarray(xqa),
            "xqb": np.ascontiguousarray(xqb),
        })
    return in_maps


def _gather(results):
    ys = np.concatenate([results[i]["y"] for i in range(N_CORES)], axis=0)
    return np.ascontiguousarray(ys[:C].transpose(1, 2, 0)).astype(np.float32)


def run(x, W, b, **run_kwargs):
    """Full pipeline, returns (output, BassKernelResults)."""
    nc = _get_module()
    in_maps = _prep_inputs(np.asarray(x), np.asarray(W), np.asarray(b))
    res = run_bass_kernel_spmd(nc, in_maps, list(range(N_CORES)), **run_kwargs)
    return _gather(res.results), res


def kernel(x, W, b):
    out, _ = run(x, W, b)
    return out
